# revision 1
# baseline (speedup 1.0000x reference)
"""Trainium2 Bass kernel for a 2-layer GraphSAGE (sum aggregation) GNN.

Strategy (8 NeuronCores, SPMD, two launches):
  - dst nodes sharded 12500/core in natural order; 98 tiles of 128.
  - Edges bucketed by (dst tile t, src range b); four 25088-row src
    ranges keep dma_gather indices in int16. Each (t,b) run is padded
    to whole 128-edge columns; pad slots carry idx=-1 (the gather
    skips them -- no DMA traffic) and code=-1 (masked by the one-hot).
  - x rows are stored split-fp16: [fp16(x) | fp16(x - fp16(x))], 512B.
    Both halves accumulate in f32 PSUM => ~f32 precision end to end.
  - Launch 1: bulk gpsimd.dma_gather (4 SWDGE queues round-robin;
    per-core valid counts come from value_load) pulls edge rows into
    SBUF edge-major; DVE builds one-hot matrices from per-edge dst
    codes; PE accumulates aggT[tile] = sum_col gbuf_hi^T @ oh +
    gbuf_lo^T @ oh in PSUM. Weight path fp32: hT = relu(W1n^T aggT +
    W1s^T xT + b1); z = h W2n, o2 = h W2s + b2 per tile.
  - Host: concat per-core z shards, split-fp16, pad rows to 256B.
  - Launch 2: same gather structure over z rows; per column one matmul
    lhsT=oh, rhs=z hi|lo (16 cols) accumulates node-major [dst, 16] in
    PSUM; hi+lo summed by DVE; + o2; fused log_softmax.
"""

import sys

import numpy as np

sys.path.insert(0, "/opt/trn_rl_repo")

import concourse.bass as bass
import concourse.mybir as mybir
import concourse.tile as tile
from concourse import bacc
from concourse.bass_utils import run_bass_kernel_spmd
from concourse.masks import make_identity

P = 128
N_NODES = 100000
N_CORES = 8
NPC = N_NODES // N_CORES  # 12500
NT = 98  # tiles per core
NR = NT * P  # 12544
NCLS = 8
RNG = 25088  # src range size (int16 index space)
NBUK = 4
ROWS = NBUK * RNG  # 100352 padded rows
B = 96  # max gather columns per batch
CPC = 8  # max columns per dma_gather call (1024-idx SWDGE ring cap)
NQ = 4  # SWDGE queues
SKIP_PADS = False  # idx=-1 pad skipping crashes the Q7 at this scale
DBG_NO_AGG_MM = False  # sim probing only
DBG_NO_GATHER = False
DBG_NO_ONEHOT = False
F32 = mybir.dt.float32
F16 = mybir.dt.float16
I16 = mybir.dt.int16
I32 = mybir.dt.int32


def _split_f16(a):
    hi = a.astype(np.float16)
    lo = (a - hi.astype(np.float32)).astype(np.float16)
    return hi, lo


def _prep(edge_src, edge_dst, Bcols=B):
    """Bucket edges by (core, tile, src-range); build the global column
    layout, per-core idx/code tensors and per-call valid counts."""
    es = np.asarray(edge_src).astype(np.int64)
    ed = np.asarray(edge_dst).astype(np.int64)
    k = ed // NPC
    d = ed - k * NPC
    t = d >> 7
    code = (d & 127).astype(np.float32)
    b = es // RNG
    il = (es - b * RNG).astype(np.int16)

    E = np.zeros((N_CORES, NT, NBUK), np.int64)
    np.add.at(E, (k, t, b), 1)
    CB = ((E + 127) // 128).max(axis=0)  # [NT, NBUK]
    tile_cols = CB.sum(axis=1)
    assert tile_cols.max() <= Bcols, tile_cols.max()

    batches = []
    cur, acc = [], 0
    for tt in range(NT):
        if acc + tile_cols[tt] > Bcols:
            batches.append(cur)
            cur, acc = [], 0
        cur.append(tt)
        acc += tile_cols[tt]
    if cur:
        batches.append(cur)

    Emax = E.max(axis=0)  # [NT, NBUK] per-run uniform valid count
    colstart = np.zeros((NT, NBUK), np.int64)
    batch_info = []  # (tiles, cb0, cb1, [(bucket, c0, c1, nvalid), ...])
    c = 0
    valid_chunks = []
    for tiles in batches:
        cb0 = c
        calls = []
        for bb in range(NBUK):
            c0 = c
            for tt in tiles:
                colstart[tt, bb] = c
                c += CB[tt, bb]
                v = np.zeros(CB[tt, bb] * P, bool)
                v[: Emax[tt, bb]] = True
                valid_chunks.append(v)
            # chunk the bucket segment into <=CPC-column calls
            cc = c0
            while cc < c:
                ce = min(cc + CPC, c)
                calls.append((bb, cc, ce))
                cc = ce
        batch_info.append((tiles, cb0, c, calls))
    TOT = c
    valid_flat = np.concatenate(valid_chunks) if valid_chunks else np.zeros(0, bool)
    assert valid_flat.size == TOT * P
    vcum = np.r_[0, np.cumsum(valid_flat)]
    # attach static per-call valid counts
    batch_info = [
        (
            tiles,
            cb0,
            cb1,
            [
                (bb, c0, c1, int(vcum[c1 * P] - vcum[c0 * P]))
                for bb, c0, c1 in calls
            ],
        )
        for tiles, cb0, cb1, calls in batch_info
    ]

    per_core = []
    for kk in range(N_CORES):
        m = k == kk
        ts, bs_, cs, ils = t[m], b[m], code[m], il[m]
        key = ts * NBUK + bs_
        o = np.argsort(key, kind="stable")
        ts, bs_, cs, ils = ts[o], bs_[o], cs[o], ils[o]
        gkey = ts * NBUK + bs_
        starts = np.r_[0, np.flatnonzero(np.diff(gkey)) + 1]
        gid = np.zeros(len(gkey), np.int64)
        gid[starts[1:]] = 1
        gid = np.cumsum(gid)
        j = np.arange(len(gkey)) - starts[gid]
        pos = colstart[ts, bs_] * P + j
        idx_flat = np.full(TOT * P, -1, np.int16)
        code_flat = np.full(TOT * P, -1.0, np.float32)
        idx_flat[valid_flat] = 0  # dummy: gather row 0, masked by code
        idx_flat[pos] = ils
        code_flat[pos] = cs
        if not SKIP_PADS:
            idx_flat[idx_flat < 0] = 0
        idx_dev = np.tile(
            np.ascontiguousarray(idx_flat.reshape(TOT * 8, 16).T), (8, 1)
        )
        codes_dev = np.ascontiguousarray(
            code_flat.reshape(TOT, P).T.astype(np.float16)
        )
        per_core.append((idx_dev, codes_dev))

    return batch_info, CB, colstart, TOT, per_core


def _gather_batch(nc, gbuf, src, idx_t, calls, cb0, qoff, elem):
    """Issue the per-batch dma_gather calls (round-robin queues).
    nvalid is core-uniform by construction, so num_idxs_reg is static."""
    q = qoff
    for bb, c0, c1, nvalid in calls:
        bb, c0, c1 = int(bb), int(c0), int(c1)
        n = (c1 - c0) * P
        nidr = int(nvalid) if SKIP_PADS else n
        if DBG_NO_GATHER:
            q[0] += 1
            q[1] += 1
            continue
        nc.gpsimd.dma_gather(
            gbuf[:, c0 - cb0 : c1 - cb0, :],
            src[bb * RNG : (bb + 1) * RNG, :],
            idx_t[:, (c0 - cb0) * 8 : (c1 - cb0) * 8],
            n,
            nidr,
            elem,
            elem_step=elem,
            queue_num=q[0] % NQ,
        )
        q[0] += 1
        q[1] += 1


def _build_nc1(batch_info, CB, colstart, TOT, Bc=B):
    nc = bacc.Bacc(
        "TRN2", target_bir_lowering=False, debug=False,
        num_devices=N_CORES, num_swdge_queues=NQ,
    )
    xs = nc.dram_tensor("xs", [ROWS, 2 * P], F16, kind="ExternalInput").ap()
    xo = nc.dram_tensor("xo", [NR, P], F32, kind="ExternalInput").ap()
    idx = nc.dram_tensor("idx", [P, TOT * 8], I16, kind="ExternalInput").ap()
    cds = nc.dram_tensor("cds", [P, TOT], F16, kind="ExternalInput").ap()
    W1n = nc.dram_tensor("W1n", [P, P], F32, kind="ExternalInput").ap()
    W1s = nc.dram_tensor("W1s", [P, P], F32, kind="ExternalInput").ap()
    W2n = nc.dram_tensor("W2n", [P, NCLS], F32, kind="ExternalInput").ap()
    W2s = nc.dram_tensor("W2s", [P, NCLS], F32, kind="ExternalInput").ap()
    b1 = nc.dram_tensor("b1", [P, 1], F32, kind="ExternalInput").ap()
    b2 = nc.dram_tensor("b2", [1, NCLS], F32, kind="ExternalInput").ap()
    z_o = nc.dram_tensor("z", [P, NT * NCLS], F32, kind="ExternalOutput").ap()
    o2_o = nc.dram_tensor("o2", [P, NT * NCLS], F32, kind="ExternalOutput").ap()

    with tile.TileContext(nc) as tc:
        with (
            tc.tile_pool(name="persist", bufs=1) as pp,
            tc.tile_pool(name="stream", bufs=2) as sp,
            tc.tile_pool(name="gather", bufs=2) as gp,
            tc.tile_pool(name="xown", bufs=3) as xp,
            tc.tile_pool(name="work", bufs=3) as wp,
            tc.tile_pool(name="psA", bufs=2, space="PSUM") as psA,
            tc.tile_pool(name="psB", bufs=2, space="PSUM") as psB,
            tc.tile_pool(name="psC", bufs=2, space="PSUM") as psC,
            tc.tile_pool(name="psD", bufs=2, space="PSUM") as psD,
        ):
            w1n = pp.tile([P, P], F32, tag="w1n")
            w1s = pp.tile([P, P], F32, tag="w1s")
            w2n = pp.tile([P, NCLS], F32, tag="w2n")
            w2s = pp.tile([P, NCLS], F32, tag="w2s")
            b1t = pp.tile([P, 1], F32, tag="b1")
            b2t = pp.tile([1, NCLS], F32, tag="b2")
            ones = pp.tile([1, P], F32, tag="ones")
            ident = pp.tile([P, P], F32, tag="ident")
            iota_i = pp.tile([P, P], I16, tag="iota_i")
            iota_h = pp.tile([P, P], F16, tag="iota_h")
            z_sb = pp.tile([P, NT * NCLS], F32, tag="z_sb")
            o2_sb = pp.tile([P, NT * NCLS], F32, tag="o2_sb")

            nc.sync.dma_start(out=w1n[:], in_=W1n[:])
            nc.sync.dma_start(out=w1s[:], in_=W1s[:])
            nc.sync.dma_start(out=w2n[:], in_=W2n[:])
            nc.sync.dma_start(out=w2s[:], in_=W2s[:])
            nc.sync.dma_start(out=b1t[:], in_=b1[:])
            nc.sync.dma_start(out=b2t[:], in_=b2[:])
            nc.vector.memset(ones[:], 1.0)
            make_identity(nc, ident[:])
            nc.gpsimd.iota(
                iota_i[:], pattern=[[1, P]], base=0, channel_multiplier=0
            )
            nc.vector.tensor_copy(out=iota_h[:], in_=iota_i[:])
            # pad slots skip the gather; stale SBUF could hold NaN and
            # 0*NaN poisons PSUM -- zero both gather ring buffers once.
            for _ in range(2):
                gz = gp.tile([P, Bc, 2 * P], F16, tag="g")
                nc.vector.memset(gz[:], 0.0)

            qoff = [0, 0]
            for tiles, cb0, cb1, calls in batch_info:
                bw = cb1 - cb0
                idx_t = sp.tile([P, Bc * 8], I16, tag="idx")
                cds_t = sp.tile([P, Bc], F16, tag="cds")
                nc.sync.dma_start(
                    out=idx_t[:, : bw * 8], in_=idx[:, cb0 * 8 : cb1 * 8]
                )
                nc.sync.dma_start(out=cds_t[:, :bw], in_=cds[:, cb0:cb1])
                gbuf = gp.tile([P, Bc, 2 * P], F16, tag="g")
                _gather_batch(
                    nc, gbuf, xs, idx_t, calls, cb0, qoff, 2 * P
                )
                oh = gp.tile([P, Bc, P], F16, tag="oh")
                if not DBG_NO_ONEHOT:
                    h1 = (bw + 1) // 2
                    for o0, o1 in ((0, h1), (h1, bw)):
                        if o1 > o0:
                            nc.vector.tensor_tensor(
                                out=oh[:, o0:o1, :],
                                in0=cds_t[:, o0:o1]
                                .unsqueeze(2)
                                .to_broadcast([P, o1 - o0, P]),
                                in1=iota_h[:]
                                .unsqueeze(1)
                                .to_broadcast([P, o1 - o0, P]),
                                op=mybir.AluOpType.is_equal,
                            )

                for t in tiles:
                    agg_ps = psA.tile([P, P], F32, tag="agg")
                    runs = [
                        (int(colstart[t, bb] - cb0), int(CB[t, bb]))
                        for bb in range(NBUK)
                        if CB[t, bb] > 0
                    ]
                    ncols = sum(n for _, n in runs)
                    ci = 0
                    for lc0, n in runs:
                        if DBG_NO_AGG_MM:
                            break
                        for c in range(lc0, lc0 + n):
                            nc.tensor.matmul(
                                out=agg_ps[:],
                                lhsT=gbuf[:, c, 0:P],
                                rhs=oh[:, c, :],
                                start=(ci == 0),
                                stop=False,
                            )
                            nc.tensor.matmul(
                                out=agg_ps[:],
                                lhsT=gbuf[:, c, P : 2 * P],
                                rhs=oh[:, c, :],
                                start=False,
                                stop=(ci == ncols - 1),
                            )
                            ci += 1
                    if DBG_NO_AGG_MM:
                        nc.tensor.matmul(
                            out=agg_ps[:], lhsT=gbuf[:, 0, 0:P],
                            rhs=oh[:, 0, :], start=True, stop=True,
                        )
                    aggT = wp.tile([P, P], F32, tag="aggT")
                    nc.scalar.activation(
                        out=aggT[:], in_=agg_ps[:],
                        func=mybir.ActivationFunctionType.Copy,
                    )

                    xot = xp.tile([P, P], F32, tag="xot")
                    nc.sync.dma_start(
                        out=xot[:], in_=xo[t * P : (t + 1) * P, :]
                    )
                    xT_ps = psB.tile([P, P], F32, tag="xT")
                    nc.tensor.transpose(
                        out=xT_ps[:], in_=xot[:], identity=ident[:]
                    )
                    xT = wp.tile([P, P], F32, tag="xT_sb")
                    nc.scalar.activation(
                        out=xT[:], in_=xT_ps[:],
                        func=mybir.ActivationFunctionType.Copy,
                    )

                    hT_ps = psC.tile([P, P], F32, tag="hT")
                    nc.tensor.matmul(
                        out=hT_ps[:], lhsT=w1n[:], rhs=aggT[:],
                        start=True, stop=False,
                    )
                    nc.tensor.matmul(
                        out=hT_ps[:], lhsT=w1s[:], rhs=xT[:],
                        start=False, stop=True,
                    )
                    hT = wp.tile([P, P], F32, tag="hT_sb")
                    nc.scalar.activation(
                        out=hT[:], in_=hT_ps[:],
                        func=mybir.ActivationFunctionType.Relu,
                        bias=b1t[:, 0:1],
                    )

                    zo_ps = psD.tile([P, 2 * NCLS], F32, tag="zo")
                    nc.tensor.matmul(
                        out=zo_ps[:, 0:NCLS], lhsT=hT[:], rhs=w2n[:],
                        start=True, stop=True,
                    )
                    nc.tensor.matmul(
                        out=zo_ps[:, NCLS : 2 * NCLS], lhsT=hT[:], rhs=w2s[:],
                        start=True, stop=False,
                    )
                    nc.tensor.matmul(
                        out=zo_ps[:, NCLS : 2 * NCLS], lhsT=ones[0:1, :],
                        rhs=b2t[0:1, :], start=False, stop=True,
                    )
                    zsl = slice(t * NCLS, (t + 1) * NCLS)
                    nc.vector.tensor_copy(
                        out=z_sb[:, zsl], in_=zo_ps[:, 0:NCLS]
                    )
                    nc.vector.tensor_copy(
                        out=o2_sb[:, zsl], in_=zo_ps[:, NCLS : 2 * NCLS]
                    )
                    # (z/o2 copies stay on DVE: ACT already busy with relu)

            nc.sync.dma_start(out=z_o, in_=z_sb[:])
            nc.sync.dma_start(out=o2_o, in_=o2_sb[:])

    nc.compile()
    return nc


def _build_nc2(batch_info, CB, colstart, TOT, Bc=B):
    nc = bacc.Bacc(
        "TRN2", target_bir_lowering=False, debug=False,
        num_devices=N_CORES, num_swdge_queues=NQ,
    )
    zs = nc.dram_tensor("zs", [ROWS, P], F16, kind="ExternalInput").ap()
    idx = nc.dram_tensor("idx", [P, TOT * 8], I16, kind="ExternalInput").ap()
    cds = nc.dram_tensor("cds", [P, TOT], F16, kind="ExternalInput").ap()
    o2_i = nc.dram_tensor("o2", [P, NT * NCLS], F32, kind="ExternalInput").ap()
    out = nc.dram_tensor("out", [P, NT * NCLS], F32, kind="ExternalOutput").ap()

    with tile.TileContext(nc) as tc:
        with (
            tc.tile_pool(name="persist", bufs=1) as pp,
            tc.tile_pool(name="stream", bufs=2) as sp,
            tc.tile_pool(name="gather", bufs=2) as gp,
            tc.tile_pool(name="work", bufs=3) as wp,
            tc.tile_pool(name="psA", bufs=4, space="PSUM") as psA,
        ):
            iota_i = pp.tile([P, P], I16, tag="iota_i")
            iota_h = pp.tile([P, P], F16, tag="iota_h")
            o2_sb = pp.tile([P, NT * NCLS], F32, tag="o2_sb")
            a2_sb = pp.tile([P, NT * NCLS], F32, tag="a2_sb")
            nc.gpsimd.iota(
                iota_i[:], pattern=[[1, P]], base=0, channel_multiplier=0
            )
            nc.vector.tensor_copy(out=iota_h[:], in_=iota_i[:])
            nc.sync.dma_start(out=o2_sb[:], in_=o2_i[:])
            for _ in range(2):
                gz = gp.tile([P, Bc, P], F16, tag="g")
                nc.vector.memset(gz[:], 0.0)

            qoff = [0, 0]
            for tiles, cb0, cb1, calls in batch_info:
                bw = cb1 - cb0
                idx_t = sp.tile([P, Bc * 8], I16, tag="idx")
                cds_t = sp.tile([P, Bc], F16, tag="cds")
                nc.sync.dma_start(
                    out=idx_t[:, : bw * 8], in_=idx[:, cb0 * 8 : cb1 * 8]
                )
                nc.sync.dma_start(out=cds_t[:, :bw], in_=cds[:, cb0:cb1])
                zbuf = gp.tile([P, Bc, P], F16, tag="g")
                _gather_batch(
                    nc, zbuf, zs, idx_t, calls, cb0, qoff, P
                )
                oh = gp.tile([P, Bc, P], F16, tag="oh")
                h1 = (bw + 1) // 2
                for o0, o1 in ((0, h1), (h1, bw)):
                    if o1 > o0:
                        nc.vector.tensor_tensor(
                            out=oh[:, o0:o1, :],
                            in0=cds_t[:, o0:o1]
                            .unsqueeze(2)
                            .to_broadcast([P, o1 - o0, P]),
                            in1=iota_h[:]
                            .unsqueeze(1)
                            .to_broadcast([P, o1 - o0, P]),
                            op=mybir.AluOpType.is_equal,
                        )

                for t in tiles:
                    a2_ps = psA.tile([P, 2 * NCLS], F32, tag="a2")
                    runs = [
                        (int(colstart[t, bb] - cb0), int(CB[t, bb]))
                        for bb in range(NBUK)
                        if CB[t, bb] > 0
                    ]
                    ncols = sum(n for _, n in runs)
                    ci = 0
                    for lc0, n in runs:
                        for c in range(lc0, lc0 + n):
                            nc.tensor.matmul(
                                out=a2_ps[:],
                                lhsT=oh[:, c, :],
                                rhs=zbuf[:, c, 0 : 2 * NCLS],
                                start=(ci == 0),
                                stop=(ci == ncols - 1),
                            )
                            ci += 1
                    zsl = slice(t * NCLS, (t + 1) * NCLS)
                    a2t = wp.tile([P, 2 * NCLS], F32, tag="a2t")
                    nc.vector.tensor_copy(out=a2t[:], in_=a2_ps[:])
                    nc.vector.tensor_tensor(
                        out=a2_sb[:, zsl],
                        in0=a2t[:, 0:NCLS],
                        in1=a2t[:, NCLS : 2 * NCLS],
                        op=mybir.AluOpType.add,
                    )

            nc.vector.tensor_add(out=a2_sb[:], in0=a2_sb[:], in1=o2_sb[:])
            a3 = a2_sb[:].rearrange("p (t c) -> p t c", c=NCLS)
            mx = pp.tile([P, NT], F32, tag="mx")
            nc.vector.tensor_reduce(
                out=mx[:], in_=a3, axis=mybir.AxisListType.X,
                op=mybir.AluOpType.max,
            )
            mxb = mx[:].unsqueeze(2).to_broadcast([P, NT, NCLS])
            nc.vector.tensor_tensor(
                out=a3, in0=a3, in1=mxb, op=mybir.AluOpType.subtract
            )
            ex = pp.tile([P, NT * NCLS], F32, tag="ex")
            nc.scalar.activation(
                out=ex[:], in_=a2_sb[:], func=mybir.ActivationFunctionType.Exp
            )
            sm = pp.tile([P, NT], F32, tag="sm")
            nc.vector.tensor_reduce(
                out=sm[:],
                in_=ex[:].rearrange("p (t c) -> p t c", c=NCLS),
                axis=mybir.AxisListType.X,
                op=mybir.AluOpType.add,
            )
            lg = pp.tile([P, NT], F32, tag="lg")
            nc.scalar.activation(
                out=lg[:], in_=sm[:], func=mybir.ActivationFunctionType.Ln
            )
            lgb = lg[:].unsqueeze(2).to_broadcast([P, NT, NCLS])
            nc.vector.tensor_tensor(
                out=a3, in0=a3, in1=lgb, op=mybir.AluOpType.subtract
            )
            nc.sync.dma_start(out=out, in_=a2_sb[:])

    nc.compile()
    return nc


_CACHE = {}


def _rows(a):
    """[P, NT*NCLS] sbuf layout -> [NR, NCLS] node-order rows."""
    return np.ascontiguousarray(
        a.reshape(P, NT, NCLS).transpose(1, 0, 2).reshape(NR, NCLS)
    )


def kernel(x, edge_src, edge_dst, W_neigh1, W_self1, b1, W_neigh2, W_self2, b2):
    x = np.ascontiguousarray(np.asarray(x, dtype=np.float32))
    B2 = 128
    batch_info, CB, colstart, TOT, per_core = _prep(edge_src, edge_dst)
    batch_info2, CB2, colstart2, TOT2, per_core2 = _prep(
        edge_src, edge_dst, Bcols=B2
    )

    xpad = np.zeros((ROWS, P), np.float32)
    xpad[:N_NODES] = x
    xh, xl = _split_f16(xpad)
    xsplit = np.ascontiguousarray(np.concatenate([xh, xl], axis=1))

    common1 = {
        "xs": xsplit,
        "W1n": np.asarray(W_neigh1, np.float32),
        "W1s": np.asarray(W_self1, np.float32),
        "W2n": np.asarray(W_neigh2, np.float32),
        "W2s": np.asarray(W_self2, np.float32),
        "b1": np.asarray(b1, np.float32).reshape(P, 1),
        "b2": np.asarray(b2, np.float32).reshape(1, NCLS),
    }
    in_maps1 = [
        {
            **common1,
            "xo": np.ascontiguousarray(xpad[k * NPC : k * NPC + NR]),
            "idx": per_core[k][0],
            "cds": per_core[k][1],
        }
        for k in range(N_CORES)
    ]

    key = ("nc", TOT, TOT2, tuple(len(t) for t, *_ in batch_info))
    if key not in _CACHE:
        _CACHE[key] = (
            _build_nc1(batch_info, CB, colstart, TOT),
            _build_nc2(batch_info2, CB2, colstart2, TOT2, Bc=B2),
        )
    nc1, nc2 = _CACHE[key]

    res1 = run_bass_kernel_spmd(nc1, in_maps1, list(range(N_CORES)))

    z_full = np.concatenate(
        [_rows(res1.results[k]["z"])[:NPC] for k in range(N_CORES)], axis=0
    )
    zh, zl = _split_f16(z_full)
    zsplit = np.zeros((ROWS, P), np.float16)
    zsplit[:N_NODES, 0:NCLS] = zh
    zsplit[:N_NODES, NCLS : 2 * NCLS] = zl

    in_maps2 = [
        {
            "zs": zsplit,
            "idx": per_core2[k][0],
            "cds": per_core2[k][1],
            "o2": res1.results[k]["o2"],
        }
        for k in range(N_CORES)
    ]
    res2 = run_bass_kernel_spmd(nc2, in_maps2, list(range(N_CORES)))

    out_full = np.empty((N_NODES, NCLS), dtype=np.float32)
    for k in range(N_CORES):
        out_full[k * NPC : (k + 1) * NPC] = _rows(res2.results[k]["out"])[:NPC]

    kernel._last = (nc1, in_maps1, nc2, in_maps2)
    return out_full


if __name__ == "__main__":
    import jax

    import reference

    cpu = jax.devices("cpu")[0]
    with jax.default_device(cpu):
        inputs = {k: np.asarray(v) for k, v in reference.setup_inputs().items()}
        exp = np.asarray(
            reference.reference(
                **{k: jax.device_put(v, cpu) for k, v in inputs.items()}
            )
        )
    got = kernel(**inputs)
    err = np.abs(got - exp)
    rel = err / np.maximum(np.abs(exp), 1e-3)
    print("max abs err:", err.max(), "max rel err:", rel.max())



# revision 11
# speedup vs baseline: 1.1652x; 1.1652x over previous
"""Trainium2 Bass kernel for a 2-layer GraphSAGE (sum aggregation) GNN.

Strategy (8 NeuronCores, SPMD, two launches):
  - dst nodes sharded 12500/core in natural order; 98 tiles of 128.
  - Edges bucketed by (dst tile t, src range b); four 25088-row src
    ranges keep dma_gather indices in int16. Each (t,b) run is padded
    to whole 128-edge columns; pad slots carry idx=0 (gathers row 0)
    and code=-1 (masked by the one-hot).
  - x rows are gathered as single fp16, 256B/row (rel err ~1.5e-2 vs
    the 2e-2 gate; x@W_self stays exact f32, and the z path is hi/lo
    so layer-2 adds ~nothing).
  - Launch 1: bulk gpsimd.dma_gather (4 SWDGE queues round-robin, one
    call per (batch, bucket) segment) pulls edge rows into SBUF
    edge-major; DVE builds one-hot matrices from per-edge dst codes;
    PE accumulates aggT[tile] = sum_col gbuf^T @ oh in PSUM. Weight
    path fp32 with host-pretransposed xT: hT = relu(W1n^T aggT +
    W1s^T xT + b1); z = h W2n, o2 = h W2s + b2 per tile.
  - Host: concat per-core z shards, split-fp16, pad rows to 256B.
  - Launch 2: same gather structure over z rows; per column one matmul
    lhsT=oh, rhs=z hi|lo (16 cols) accumulates node-major [dst, 16] in
    PSUM; hi+lo summed by DVE; + o2; fused log_softmax.
"""

import sys

import numpy as np

sys.path.insert(0, "/opt/trn_rl_repo")

import concourse.bass as bass
import concourse.mybir as mybir
import concourse.tile as tile
from concourse import bacc
from concourse.bass_utils import run_bass_kernel_spmd
from concourse.masks import make_identity

P = 128
N_NODES = 100000
N_CORES = 8
NPC = N_NODES // N_CORES  # 12500
NT = 98  # tiles per core
NR = NT * P  # 12544
NCLS = 8
RNG = 25088  # src range size (int16 index space)
NBUK = 4
ROWS = NBUK * RNG  # 100352 padded rows
B = 96  # max gather columns per batch
CPC = 8  # max columns per dma_gather call (1024-idx SWDGE ring cap;
# 32-col calls crash the Q7 on hardware -- do not raise)
NQ = 4  # SWDGE queues
SKIP_PADS = False  # idx=-1 pad skipping crashes the Q7 at this scale
DBG_NO_AGG_MM = False  # sim probing only
DBG_NO_GATHER = False
DBG_NO_ONEHOT = False
F32 = mybir.dt.float32
F16 = mybir.dt.float16
I16 = mybir.dt.int16
I32 = mybir.dt.int32


def _split_f16(a):
    hi = a.astype(np.float16)
    lo = (a - hi.astype(np.float32)).astype(np.float16)
    return hi, lo


def _prep(edge_src, edge_dst, Bcols=B):
    """Bucket edges by (core, tile, src-range); build the global column
    layout, per-core idx/code tensors and per-call valid counts."""
    es = np.asarray(edge_src).astype(np.int64)
    ed = np.asarray(edge_dst).astype(np.int64)
    k = ed // NPC
    d = ed - k * NPC
    t = d >> 7
    code = (d & 127).astype(np.float32)
    b = es // RNG
    il = (es - b * RNG).astype(np.int16)

    E = np.zeros((N_CORES, NT, NBUK), np.int64)
    np.add.at(E, (k, t, b), 1)
    CB = ((E + 127) // 128).max(axis=0)  # [NT, NBUK]
    tile_cols = CB.sum(axis=1)
    assert tile_cols.max() <= Bcols, tile_cols.max()

    batches = []
    cur, acc = [], 0
    for tt in range(NT):
        if acc + tile_cols[tt] > Bcols:
            batches.append(cur)
            cur, acc = [], 0
        cur.append(tt)
        acc += tile_cols[tt]
    if cur:
        batches.append(cur)

    Emax = E.max(axis=0)  # [NT, NBUK] per-run uniform valid count
    colstart = np.zeros((NT, NBUK), np.int64)
    batch_info = []  # (tiles, cb0, cb1, [(bucket, c0, c1, nvalid), ...])
    c = 0
    valid_chunks = []
    for tiles in batches:
        cb0 = c
        calls = []
        for bb in range(NBUK):
            c0 = c
            for tt in tiles:
                colstart[tt, bb] = c
                c += CB[tt, bb]
                v = np.zeros(CB[tt, bb] * P, bool)
                v[: Emax[tt, bb]] = True
                valid_chunks.append(v)
            # chunk the bucket segment into <=CPC-column calls
            cc = c0
            while cc < c:
                ce = min(cc + CPC, c)
                calls.append((bb, cc, ce))
                cc = ce
        batch_info.append((tiles, cb0, c, calls))
    TOT = c
    valid_flat = np.concatenate(valid_chunks) if valid_chunks else np.zeros(0, bool)
    assert valid_flat.size == TOT * P
    vcum = np.r_[0, np.cumsum(valid_flat)]
    # attach static per-call valid counts
    batch_info = [
        (
            tiles,
            cb0,
            cb1,
            [
                (bb, c0, c1, int(vcum[c1 * P] - vcum[c0 * P]))
                for bb, c0, c1 in calls
            ],
        )
        for tiles, cb0, cb1, calls in batch_info
    ]

    per_core = []
    for kk in range(N_CORES):
        m = k == kk
        ts, bs_, cs, ils = t[m], b[m], code[m], il[m]
        key = ts * NBUK + bs_
        o = np.argsort(key, kind="stable")
        ts, bs_, cs, ils = ts[o], bs_[o], cs[o], ils[o]
        gkey = ts * NBUK + bs_
        starts = np.r_[0, np.flatnonzero(np.diff(gkey)) + 1]
        gid = np.zeros(len(gkey), np.int64)
        gid[starts[1:]] = 1
        gid = np.cumsum(gid)
        j = np.arange(len(gkey)) - starts[gid]
        pos = colstart[ts, bs_] * P + j
        idx_flat = np.full(TOT * P, -1, np.int16)
        code_flat = np.full(TOT * P, -1.0, np.float32)
        idx_flat[valid_flat] = 0  # dummy: gather row 0, masked by code
        idx_flat[pos] = ils
        code_flat[pos] = cs
        if not SKIP_PADS:
            idx_flat[idx_flat < 0] = 0
        idx_dev = np.tile(
            np.ascontiguousarray(idx_flat.reshape(TOT * 8, 16).T), (8, 1)
        )
        codes_dev = np.ascontiguousarray(
            code_flat.reshape(TOT, P).T.astype(np.float16)
        )
        per_core.append((idx_dev, codes_dev))

    return batch_info, CB, colstart, TOT, per_core


def _gather_batch(nc, gbuf, src, idx_t, calls, cb0, qoff, elem):
    """Issue the per-batch dma_gather calls (round-robin queues).
    nvalid is core-uniform by construction, so num_idxs_reg is static."""
    q = qoff
    for bb, c0, c1, nvalid in calls:
        bb, c0, c1 = int(bb), int(c0), int(c1)
        n = (c1 - c0) * P
        nidr = int(nvalid) if SKIP_PADS else n
        if DBG_NO_GATHER:
            q[0] += 1
            q[1] += 1
            continue
        nc.gpsimd.dma_gather(
            gbuf[:, c0 - cb0 : c1 - cb0, :],
            src[bb * RNG : (bb + 1) * RNG, :],
            idx_t[:, (c0 - cb0) * 8 : (c1 - cb0) * 8],
            n,
            nidr,
            elem,
            elem_step=elem,
            queue_num=q[0] % NQ,
        )
        q[0] += 1
        q[1] += 1


def _build_nc1(batch_info, CB, colstart, TOT, Bc=B):
    nc = bacc.Bacc(
        "TRN2", target_bir_lowering=False, debug=False,
        num_devices=N_CORES, num_swdge_queues=NQ,
    )
    xs = nc.dram_tensor("xs", [ROWS, P], F16, kind="ExternalInput").ap()
    xoT = nc.dram_tensor("xoT", [P, NT * P], F32, kind="ExternalInput").ap()
    idx = nc.dram_tensor("idx", [P, TOT * 8], I16, kind="ExternalInput").ap()
    cds = nc.dram_tensor("cds", [P, TOT], F16, kind="ExternalInput").ap()
    W1n = nc.dram_tensor("W1n", [P, P], F32, kind="ExternalInput").ap()
    W1s = nc.dram_tensor("W1s", [P, P], F32, kind="ExternalInput").ap()
    W2n = nc.dram_tensor("W2n", [P, NCLS], F32, kind="ExternalInput").ap()
    W2s = nc.dram_tensor("W2s", [P, NCLS], F32, kind="ExternalInput").ap()
    b1 = nc.dram_tensor("b1", [P, 1], F32, kind="ExternalInput").ap()
    b2 = nc.dram_tensor("b2", [1, NCLS], F32, kind="ExternalInput").ap()
    z_o = nc.dram_tensor("z", [P, NT * NCLS], F32, kind="ExternalOutput").ap()
    o2_o = nc.dram_tensor("o2", [P, NT * NCLS], F32, kind="ExternalOutput").ap()

    with tile.TileContext(nc) as tc:
        with (
            tc.tile_pool(name="persist", bufs=1) as pp,
            tc.tile_pool(name="stream", bufs=2) as sp,
            tc.tile_pool(name="gather", bufs=2) as gp,
            tc.tile_pool(name="xown", bufs=3) as xp,
            tc.tile_pool(name="work", bufs=3) as wp,
            tc.tile_pool(name="psA", bufs=2, space="PSUM") as psA,
            tc.tile_pool(name="psC", bufs=2, space="PSUM") as psC,
            tc.tile_pool(name="psD", bufs=2, space="PSUM") as psD,
        ):
            w1n = pp.tile([P, P], F32, tag="w1n")
            w1s = pp.tile([P, P], F32, tag="w1s")
            w2n = pp.tile([P, NCLS], F32, tag="w2n")
            w2s = pp.tile([P, NCLS], F32, tag="w2s")
            b1t = pp.tile([P, 1], F32, tag="b1")
            b2t = pp.tile([1, NCLS], F32, tag="b2")
            ones = pp.tile([1, P], F32, tag="ones")
            iota_i = pp.tile([P, P], I16, tag="iota_i")
            iota_h = pp.tile([P, P], F16, tag="iota_h")
            z_sb = pp.tile([P, NT * NCLS], F32, tag="z_sb")
            o2_sb = pp.tile([P, NT * NCLS], F32, tag="o2_sb")

            nc.sync.dma_start(out=w1n[:], in_=W1n[:])
            nc.sync.dma_start(out=w1s[:], in_=W1s[:])
            nc.sync.dma_start(out=w2n[:], in_=W2n[:])
            nc.sync.dma_start(out=w2s[:], in_=W2s[:])
            nc.sync.dma_start(out=b1t[:], in_=b1[:])
            nc.sync.dma_start(out=b2t[:], in_=b2[:])
            nc.vector.memset(ones[:], 1.0)
            nc.gpsimd.iota(
                iota_i[:], pattern=[[1, P]], base=0, channel_multiplier=0
            )
            nc.vector.tensor_copy(out=iota_h[:], in_=iota_i[:])
            # stale SBUF in never-gathered pad columns could hold NaN and
            # 0*NaN poisons PSUM -- zero both gather ring buffers once.
            for _ in range(2):
                gz = gp.tile([P, Bc, P], F16, tag="g")
                nc.vector.memset(gz[:], 0.0)

            qoff = [0, 0]
            for tiles, cb0, cb1, calls in batch_info:
                bw = cb1 - cb0
                idx_t = sp.tile([P, Bc * 8], I16, tag="idx")
                cds_t = sp.tile([P, Bc], F16, tag="cds")
                nc.sync.dma_start(
                    out=idx_t[:, : bw * 8], in_=idx[:, cb0 * 8 : cb1 * 8]
                )
                nc.sync.dma_start(out=cds_t[:, :bw], in_=cds[:, cb0:cb1])
                gbuf = gp.tile([P, Bc, P], F16, tag="g")
                _gather_batch(
                    nc, gbuf, xs, idx_t, calls, cb0, qoff, P
                )
                oh = gp.tile([P, Bc, P], F16, tag="oh")
                if not DBG_NO_ONEHOT:
                    h1 = (bw + 1) // 2
                    for o0, o1 in ((0, h1), (h1, bw)):
                        if o1 > o0:
                            nc.vector.tensor_tensor(
                                out=oh[:, o0:o1, :],
                                in0=cds_t[:, o0:o1]
                                .unsqueeze(2)
                                .to_broadcast([P, o1 - o0, P]),
                                in1=iota_h[:]
                                .unsqueeze(1)
                                .to_broadcast([P, o1 - o0, P]),
                                op=mybir.AluOpType.is_equal,
                            )

                for t in tiles:
                    agg_ps = psA.tile([P, P], F32, tag="agg")
                    runs = [
                        (int(colstart[t, bb] - cb0), int(CB[t, bb]))
                        for bb in range(NBUK)
                        if CB[t, bb] > 0
                    ]
                    ncols = sum(n for _, n in runs)
                    ci = 0
                    for lc0, n in runs:
                        if DBG_NO_AGG_MM:
                            break
                        for c in range(lc0, lc0 + n):
                            nc.tensor.matmul(
                                out=agg_ps[:],
                                lhsT=gbuf[:, c, :],
                                rhs=oh[:, c, :],
                                start=(ci == 0),
                                stop=(ci == ncols - 1),
                            )
                            ci += 1
                    if DBG_NO_AGG_MM:
                        nc.tensor.matmul(
                            out=agg_ps[:], lhsT=gbuf[:, 0, :],
                            rhs=oh[:, 0, :], start=True, stop=True,
                        )
                    aggT = wp.tile([P, P], F32, tag="aggT")
                    nc.scalar.activation(
                        out=aggT[:], in_=agg_ps[:],
                        func=mybir.ActivationFunctionType.Copy,
                    )

                    xT = xp.tile([P, P], F32, tag="xT_sb")
                    nc.sync.dma_start(
                        out=xT[:], in_=xoT[:, t * P : (t + 1) * P]
                    )

                    hT_ps = psC.tile([P, P], F32, tag="hT")
                    nc.tensor.matmul(
                        out=hT_ps[:], lhsT=w1n[:], rhs=aggT[:],
                        start=True, stop=False,
                    )
                    nc.tensor.matmul(
                        out=hT_ps[:], lhsT=w1s[:], rhs=xT[:],
                        start=False, stop=True,
                    )
                    hT = wp.tile([P, P], F32, tag="hT_sb")
                    nc.scalar.activation(
                        out=hT[:], in_=hT_ps[:],
                        func=mybir.ActivationFunctionType.Relu,
                        bias=b1t[:, 0:1],
                    )

                    zo_ps = psD.tile([P, 2 * NCLS], F32, tag="zo")
                    nc.tensor.matmul(
                        out=zo_ps[:, 0:NCLS], lhsT=hT[:], rhs=w2n[:],
                        start=True, stop=True,
                    )
                    nc.tensor.matmul(
                        out=zo_ps[:, NCLS : 2 * NCLS], lhsT=hT[:], rhs=w2s[:],
                        start=True, stop=False,
                    )
                    nc.tensor.matmul(
                        out=zo_ps[:, NCLS : 2 * NCLS], lhsT=ones[0:1, :],
                        rhs=b2t[0:1, :], start=False, stop=True,
                    )
                    zsl = slice(t * NCLS, (t + 1) * NCLS)
                    nc.vector.tensor_copy(
                        out=z_sb[:, zsl], in_=zo_ps[:, 0:NCLS]
                    )
                    nc.vector.tensor_copy(
                        out=o2_sb[:, zsl], in_=zo_ps[:, NCLS : 2 * NCLS]
                    )
                    # (z/o2 copies stay on DVE: ACT already busy with relu)

            nc.sync.dma_start(out=z_o, in_=z_sb[:])
            nc.sync.dma_start(out=o2_o, in_=o2_sb[:])

    nc.compile()
    return nc


def _build_nc2(batch_info, CB, colstart, TOT, Bc=B):
    nc = bacc.Bacc(
        "TRN2", target_bir_lowering=False, debug=False,
        num_devices=N_CORES, num_swdge_queues=NQ,
    )
    zs = nc.dram_tensor("zs", [ROWS, P], F16, kind="ExternalInput").ap()
    idx = nc.dram_tensor("idx", [P, TOT * 8], I16, kind="ExternalInput").ap()
    cds = nc.dram_tensor("cds", [P, TOT], F16, kind="ExternalInput").ap()
    o2_i = nc.dram_tensor("o2", [P, NT * NCLS], F32, kind="ExternalInput").ap()
    out = nc.dram_tensor("out", [P, NT * NCLS], F32, kind="ExternalOutput").ap()

    with tile.TileContext(nc) as tc:
        with (
            tc.tile_pool(name="persist", bufs=1) as pp,
            tc.tile_pool(name="stream", bufs=2) as sp,
            tc.tile_pool(name="gather", bufs=2) as gp,
            tc.tile_pool(name="work", bufs=3) as wp,
            tc.tile_pool(name="psA", bufs=4, space="PSUM") as psA,
        ):
            iota_i = pp.tile([P, P], I16, tag="iota_i")
            iota_h = pp.tile([P, P], F16, tag="iota_h")
            o2_sb = pp.tile([P, NT * NCLS], F32, tag="o2_sb")
            a2_sb = pp.tile([P, NT * NCLS], F32, tag="a2_sb")
            nc.gpsimd.iota(
                iota_i[:], pattern=[[1, P]], base=0, channel_multiplier=0
            )
            nc.vector.tensor_copy(out=iota_h[:], in_=iota_i[:])
            nc.sync.dma_start(out=o2_sb[:], in_=o2_i[:])
            for _ in range(2):
                gz = gp.tile([P, Bc, P], F16, tag="g")
                nc.vector.memset(gz[:], 0.0)

            qoff = [0, 0]
            for tiles, cb0, cb1, calls in batch_info:
                bw = cb1 - cb0
                idx_t = sp.tile([P, Bc * 8], I16, tag="idx")
                cds_t = sp.tile([P, Bc], F16, tag="cds")
                nc.sync.dma_start(
                    out=idx_t[:, : bw * 8], in_=idx[:, cb0 * 8 : cb1 * 8]
                )
                nc.sync.dma_start(out=cds_t[:, :bw], in_=cds[:, cb0:cb1])
                zbuf = gp.tile([P, Bc, P], F16, tag="g")
                _gather_batch(
                    nc, zbuf, zs, idx_t, calls, cb0, qoff, P
                )
                oh = gp.tile([P, Bc, P], F16, tag="oh")
                h1 = (bw + 1) // 2
                for o0, o1 in ((0, h1), (h1, bw)):
                    if o1 > o0:
                        nc.vector.tensor_tensor(
                            out=oh[:, o0:o1, :],
                            in0=cds_t[:, o0:o1]
                            .unsqueeze(2)
                            .to_broadcast([P, o1 - o0, P]),
                            in1=iota_h[:]
                            .unsqueeze(1)
                            .to_broadcast([P, o1 - o0, P]),
                            op=mybir.AluOpType.is_equal,
                        )

                for t in tiles:
                    a2_ps = psA.tile([P, 2 * NCLS], F32, tag="a2")
                    runs = [
                        (int(colstart[t, bb] - cb0), int(CB[t, bb]))
                        for bb in range(NBUK)
                        if CB[t, bb] > 0
                    ]
                    ncols = sum(n for _, n in runs)
                    ci = 0
                    for lc0, n in runs:
                        for c in range(lc0, lc0 + n):
                            nc.tensor.matmul(
                                out=a2_ps[:],
                                lhsT=oh[:, c, :],
                                rhs=zbuf[:, c, 0 : 2 * NCLS],
                                start=(ci == 0),
                                stop=(ci == ncols - 1),
                            )
                            ci += 1
                    zsl = slice(t * NCLS, (t + 1) * NCLS)
                    a2t = wp.tile([P, 2 * NCLS], F32, tag="a2t")
                    nc.vector.tensor_copy(out=a2t[:], in_=a2_ps[:])
                    nc.vector.tensor_tensor(
                        out=a2_sb[:, zsl],
                        in0=a2t[:, 0:NCLS],
                        in1=a2t[:, NCLS : 2 * NCLS],
                        op=mybir.AluOpType.add,
                    )

            nc.vector.tensor_add(out=a2_sb[:], in0=a2_sb[:], in1=o2_sb[:])
            a3 = a2_sb[:].rearrange("p (t c) -> p t c", c=NCLS)
            mx = pp.tile([P, NT], F32, tag="mx")
            nc.vector.tensor_reduce(
                out=mx[:], in_=a3, axis=mybir.AxisListType.X,
                op=mybir.AluOpType.max,
            )
            mxb = mx[:].unsqueeze(2).to_broadcast([P, NT, NCLS])
            nc.vector.tensor_tensor(
                out=a3, in0=a3, in1=mxb, op=mybir.AluOpType.subtract
            )
            ex = pp.tile([P, NT * NCLS], F32, tag="ex")
            nc.scalar.activation(
                out=ex[:], in_=a2_sb[:], func=mybir.ActivationFunctionType.Exp
            )
            sm = pp.tile([P, NT], F32, tag="sm")
            nc.vector.tensor_reduce(
                out=sm[:],
                in_=ex[:].rearrange("p (t c) -> p t c", c=NCLS),
                axis=mybir.AxisListType.X,
                op=mybir.AluOpType.add,
            )
            lg = pp.tile([P, NT], F32, tag="lg")
            nc.scalar.activation(
                out=lg[:], in_=sm[:], func=mybir.ActivationFunctionType.Ln
            )
            lgb = lg[:].unsqueeze(2).to_broadcast([P, NT, NCLS])
            nc.vector.tensor_tensor(
                out=a3, in0=a3, in1=lgb, op=mybir.AluOpType.subtract
            )
            nc.sync.dma_start(out=out, in_=a2_sb[:])

    nc.compile()
    return nc


_CACHE = {}


def _rows(a):
    """[P, NT*NCLS] sbuf layout -> [NR, NCLS] node-order rows."""
    return np.ascontiguousarray(
        a.reshape(P, NT, NCLS).transpose(1, 0, 2).reshape(NR, NCLS)
    )


def kernel(x, edge_src, edge_dst, W_neigh1, W_self1, b1, W_neigh2, W_self2, b2):
    x = np.ascontiguousarray(np.asarray(x, dtype=np.float32))
    B2 = 128
    batch_info, CB, colstart, TOT, per_core = _prep(edge_src, edge_dst)
    batch_info2, CB2, colstart2, TOT2, per_core2 = _prep(
        edge_src, edge_dst, Bcols=B2
    )

    xpad = np.zeros((ROWS, P), np.float32)
    xpad[:N_NODES] = x
    x16 = np.ascontiguousarray(xpad.astype(np.float16))

    common1 = {
        "xs": x16,
        "W1n": np.asarray(W_neigh1, np.float32),
        "W1s": np.asarray(W_self1, np.float32),
        "W2n": np.asarray(W_neigh2, np.float32),
        "W2s": np.asarray(W_self2, np.float32),
        "b1": np.asarray(b1, np.float32).reshape(P, 1),
        "b2": np.asarray(b2, np.float32).reshape(1, NCLS),
    }
    in_maps1 = [
        {
            **common1,
            "xoT": np.ascontiguousarray(
                xpad[k * NPC : k * NPC + NR]
                .reshape(NT, P, P)
                .transpose(2, 0, 1)
                .reshape(P, NT * P)
            ),
            "idx": per_core[k][0],
            "cds": per_core[k][1],
        }
        for k in range(N_CORES)
    ]

    key = ("nc", TOT, TOT2, tuple(len(t) for t, *_ in batch_info))
    if key not in _CACHE:
        _CACHE[key] = (
            _build_nc1(batch_info, CB, colstart, TOT),
            _build_nc2(batch_info2, CB2, colstart2, TOT2, Bc=B2),
        )
    nc1, nc2 = _CACHE[key]

    res1 = run_bass_kernel_spmd(nc1, in_maps1, list(range(N_CORES)))

    z_full = np.concatenate(
        [_rows(res1.results[k]["z"])[:NPC] for k in range(N_CORES)], axis=0
    )
    zh, zl = _split_f16(z_full)
    zsplit = np.zeros((ROWS, P), np.float16)
    zsplit[:N_NODES, 0:NCLS] = zh
    zsplit[:N_NODES, NCLS : 2 * NCLS] = zl

    in_maps2 = [
        {
            "zs": zsplit,
            "idx": per_core2[k][0],
            "cds": per_core2[k][1],
            "o2": res1.results[k]["o2"],
        }
        for k in range(N_CORES)
    ]
    res2 = run_bass_kernel_spmd(nc2, in_maps2, list(range(N_CORES)))

    out_full = np.empty((N_NODES, NCLS), dtype=np.float32)
    for k in range(N_CORES):
        out_full[k * NPC : (k + 1) * NPC] = _rows(res2.results[k]["out"])[:NPC]

    kernel._last = (nc1, in_maps1, nc2, in_maps2)
    return out_full


if __name__ == "__main__":
    import jax

    import reference

    cpu = jax.devices("cpu")[0]
    with jax.default_device(cpu):
        inputs = {k: np.asarray(v) for k, v in reference.setup_inputs().items()}
        exp = np.asarray(
            reference.reference(
                **{k: jax.device_put(v, cpu) for k, v in inputs.items()}
            )
        )
    got = kernel(**inputs)
    err = np.abs(got - exp)
    rel = err / np.maximum(np.abs(exp), 1e-3)
    print("max abs err:", err.max(), "max rel err:", rel.max())

